# revision 13
# baseline (speedup 1.0000x reference)
"""Trainium2 Bass kernel for a 3-cell LSTM decoder step (nn_AR_Decoder).

reference semantics (per sample):
    in_frame = kps @ W_emb.T + b_emb
    h0n,c0n = LSTMCell(in_frame, h0, c0; w_ih1,w_hh1,b_ih1,b_hh1)
    h1n,c1n = LSTMCell(h0,       h1, c1; w_ih2,w_hh2,b_ih2,b_hh2)
    h2n,c2n = LSTMCell(h1,       h2, c2; w_ih3,w_hh3,b_ih3,b_hh3)
    return (h2n, h0n, h1n, h2n, c0n, c1n, c2n)

Strategy: data-parallel over 8 NeuronCores (512 samples each). On device
everything lives in a transposed [feature, batch] layout so that
  * matmuls put the contraction dim (input features) on SBUF partitions,
    weights are the stationary operand, activations stream with N=512,
  * the per-gate bias is a per-partition scalar fused into the ACT pass,
  * no transposes are needed on device (host transposes in/out instead).
Weight columns are permuted on host so the 4 gates of one 128-row h-chunk
are contiguous; each group of 4 PSUM banks then yields i/f/g/o for one
h-chunk, which the DVE combines into c_new/h_new.
Matmuls run in bf16 (fp32 PSUM accumulation).

The kernel is PE-bound: 1344 matmuls x 512 moving rows = 287us at
2.4 GHz, so the whole game is keeping the PE gap-free from the first
instruction on.  Two HW behaviours dominate the schedule:
  * the PE clock ramps 0.65 -> 1.2 -> 2.4 GHz and reaches full speed
    only after ~3.4us of CONTINUOUS activity -- any DMA-starved gap
    resets the ramp, so the startup DMA order is arranged so the PE
    starts at ~1.5us and never stalls afterwards;
  * HAM grants roughly ~297us of full-rate PE activity before clamping
    utilization to 50%, so PE busy time (incl. any warm-up) must stay
    under that budget -- hence no dummy warm-up matmuls at all.
All activation/state tensors are host-packed as [128, kc, B] so every
DMA is fully contiguous per partition (1KB-segmented DMAs run at ~half
HBM bandwidth and would stretch the critical startup window).
"""

import sys

import numpy as np

for _p in ("/opt/trn_rl_repo", "/root/.axon_site/_ro/trn_rl_repo"):
    if _p not in sys.path:
        sys.path.append(_p)

import ml_dtypes

N_CORES = 8
B_FULL = 4096
B = B_FULL // N_CORES  # 512 per core
KPS = 256
HID = 1024
G4 = 4 * HID
P = 128
KC_KPS = KPS // P  # 2
KC_HID = HID // P  # 8
NJ = KC_HID  # 8 h-chunks per cell
BF16 = ml_dtypes.bfloat16

_CACHE = {}


def _build_bass():
    import concourse.tile as tile
    from concourse import bacc, mybir

    f32 = mybir.dt.float32
    bf16 = mybir.dt.bfloat16
    AF = mybir.ActivationFunctionType

    nc = bacc.Bacc("TRN2", target_bir_lowering=False, debug=False,
                   num_devices=N_CORES)

    # ---- DRAM parameters (per-core shapes) ----
    # activations/state come in pre-packed as [P, kc, B] (host does the
    # (kc p) b -> p kc b shuffle) so every load is one fully-contiguous
    # run per partition instead of kc separate 1KB segments.
    kpsT = nc.dram_tensor("kpsT", [P, KC_KPS, B], bf16,
                          kind="ExternalInput").ap()
    hT = [nc.dram_tensor(f"h{l}T", [P, KC_HID, B], bf16,
                         kind="ExternalInput").ap() for l in range(3)]
    cT = [nc.dram_tensor(f"c{l}T", [HID, B], f32, kind="ExternalInput").ap()
          for l in range(3)]
    kcx_l = [KC_KPS, KC_HID, KC_HID]
    # group-major weight layout: wgrp[j, p, k, :] holds the [128, n_k, 512]
    # weight block of h-chunk group j exactly as the SBUF tile wants it, so
    # each group is ONE fully-contiguous DMA (n_k*1KB per partition) instead
    # of n_k 1KB segments — ~2x DMA efficiency and a much shorter startup ramp
    wgrp = [nc.dram_tensor(f"wgrp{l}", [NJ, P, kcx_l[l] + KC_HID, 4 * P],
                           bf16, kind="ExternalInput").ap() for l in range(3)]
    bias = [nc.dram_tensor(f"b{l}", [P, 4 * NJ], f32,
                           kind="ExternalInput").ap() for l in range(3)]
    # outputs in bf16: halves store DMA bytes and the final write-receipt
    # latency on the critical tail (rel-err budget has 4x headroom)
    hoT = [nc.dram_tensor(f"h{l}nT", [HID, B], bf16, kind="ExternalOutput").ap()
           for l in range(3)]
    coT = [nc.dram_tensor(f"c{l}nT", [HID, B], bf16, kind="ExternalOutput").ap()
           for l in range(3)]

    with tile.TileContext(nc) as tc:
        with (
            tc.tile_pool(name="acts", bufs=1) as acts_pool,
            tc.tile_pool(name="wpool", bufs=3) as wpool,
            tc.tile_pool(name="cpool", bufs=3) as cpool,
            tc.tile_pool(name="gates", bufs=2) as gates_pool,
            tc.tile_pool(name="ew", bufs=2) as ew_pool,
            tc.tile_pool(name="psum", bufs=8, space="PSUM") as psum_pool,
        ):
            # ---- persistent SBUF tiles ----
            kps_sb = acts_pool.tile([P, KC_KPS, B], bf16)
            h_sb = []
            for l in range(3):
                t = acts_pool.tile([P, KC_HID, B], bf16, name=f"h{l}sb")
                h_sb.append(t)
            bias_sb = []
            for l in range(3):
                t = acts_pool.tile([P, 4 * NJ], f32, name=f"b{l}sb")
                nc.scalar.dma_start(t[:], bias[l])
                bias_sb.append(t)

            # ---- HAM warm-up during the startup DMA window ----
            # The exec-time window opens ~5us before the first weight bytes
            # can land (NTFF first_useful pins to the framework memsets, and
            # the first DMA has ~1.5us HWDGE first-byte latency).  The PE is
            # idle in that window, so full-duty dummy matmuls on a garbage
            # SBUF tile are FREE there and start the HAM busy streak ~5us
            # earlier: the 8/8 grant (86 epochs) fires around the time the
            # first real matmul issues instead of ~4.5us into the real
            # stream, saving ~2us of half-clock tax.  (The earlier "net
            # zero" warm-up experiment gated the dummies on the kps arrival,
            # so they ran INSIDE the real stream and displaced real matmuls
            # 1:1; these run in otherwise-dead time.)
            warm_in = acts_pool.tile([P, P], bf16, name="warm_in")
            # gpsimd is idle at body start; the tile must be written once
            # before the PE may read it (tile allocation = first write)
            nc.gpsimd.memset(warm_in[:], 0.0)
            warm_ps = [psum_pool.tile([P, B], f32, tag="ps", name=f"warmps{q}")
                       for q in range(4)]
            for i in range(44):
                # 128-col dummies (107ns each cold) until ~the time the
                # first real matmul's data can have landed (~12.3us): keeps
                # the HAM busy streak unbroken from ~7.5us so the 8/8 grant
                # fires around the real stream start instead of ~4.5us in.
                nc.tensor.matmul(warm_ps[i % 4][:, 0:P], warm_in[:],
                                 warm_in[:], start=True, stop=True)

            # ---- startup DMA order (single sync FIFO = strict priority) ---
            # The first matmul needs only kps + group-0 k0/k1 weights
            # (~0.5MB), so those go first in ~2KB/partition pieces; h0
            # quarters are interleaved right before the k-chunks that
            # consume them.  Weights/x ride ONE queue: the early supply is
            # DMA-ISSUE-rate bound (~0.7us per 128-partition piece, i.e.
            # ~bytes/0.7us of supply), so >=256KB pieces and strict FIFO
            # priority beat both finer pieces and parallel queues.  Only the
            # non-urgent h1/h2/bias loads ride the scalar queue, so they
            # never sit in front of a weight group.
            nc.sync.dma_start(kps_sb[:], kpsT)
            nc.scalar.dma_start(h_sb[1][:], hT[1])
            nc.scalar.dma_start(h_sb[2][:], hT[2])

            # ---- the 3 LSTM cells ----
            # (x source tile, #x k-chunks, h source tile) per cell
            specs = [
                (kps_sb, KC_KPS, h_sb[0]),
                (h_sb[0], KC_HID, h_sb[1]),
                (h_sb[1], KC_HID, h_sb[2]),
            ]
            for l, (xsb, kcx, hsb) in enumerate(specs):
                n_k = kcx + KC_HID
                cr = cT[l].rearrange("(kc p) b -> p kc b", p=P)
                hor = hoT[l].rearrange("(kc p) b -> p kc b", p=P)
                cor = coT[l].rearrange("(kc p) b -> p kc b", p=P)
                for j in range(NJ):
                    wg = wpool.tile([P, n_k, 4 * P], bf16, tag="wg",
                                    name=f"wg_{l}_{j}")
                    if l == 0 and j == 0:
                        # j0 in 2-k-chunk pieces interleaved with h0
                        # quarters: the PE starts on k0/k1 (the kps chunks)
                        # after ~0.5MB of DMA and each later piece lands
                        # well before the PE reaches it, so the clock ramp
                        # is never reset by a stall.
                        nc.sync.dma_start(wg[:, 0:2, :], wgrp[l][j, :, 0:2, :])
                        for q in range(4):
                            nc.sync.dma_start(h_sb[0][:, 2 * q:2 * q + 2, :],
                                              hT[0][:, 2 * q:2 * q + 2, :])
                            nc.sync.dma_start(
                                wg[:, 2 * q + 2:2 * q + 4, :],
                                wgrp[l][j, :, 2 * q + 2:2 * q + 4, :])
                    elif l == 0 and j == 1:
                        # j1 front-loaded in small pieces too: its first
                        # k-chunks are needed ~1.5us before the whole-group
                        # DMA could have finished.
                        nc.sync.dma_start(wg[:, 0:2, :], wgrp[l][j, :, 0:2, :])
                        nc.sync.dma_start(wg[:, 2:4, :], wgrp[l][j, :, 2:4, :])
                        nc.sync.dma_start(wg[:, 4:, :], wgrp[l][j, :, 4:, :])
                    else:
                        nc.sync.dma_start(wg[:], wgrp[l][j])
                    # c rides the same FIFO right behind its group's weights
                    # (needed ~13us later, so never on the critical path, but
                    # on any other queue it would float to kernel start)
                    c_sb = cpool.tile([P, B], f32, tag="c", name=f"c_{l}_{j}")
                    nc.sync.dma_start(c_sb[:], cr[:, j, :])

                    # the very last group is split into two batch halves so
                    # its ACT/DVE/store chain overlaps the second half's
                    # matmuls instead of all trailing after the final MM.
                    # Exactly two halves is the measured optimum: no split
                    # costs ~1.8us (tail after the final matmul adds to
                    # exec 1:1), while half+quarter+quarter costs ~1us
                    # (extra instruction-fetch pressure + smaller tiles).
                    last_group = (l == 2 and j == NJ - 1)
                    parts = [(0, B // 2), (B // 2, B // 2)] if last_group \
                        else [(0, B)]
                    for b0, bw in parts:
                        bs = slice(b0, b0 + bw)
                        pss = [psum_pool.tile([P, bw], f32, tag="ps",
                                              name=f"ps{q}_{l}_{j}_{b0}")
                               for q in range(4)]
                        # k-outer / q-inner: consecutive matmuls rotate
                        # across 4 PSUM banks, hiding the PE drain-to-PSUM
                        # latency (same-bank back-to-back accumulation costs
                        # ~43ns/MM extra).
                        for k in range(n_k):
                            rhs = xsb[:, k, bs] if k < kcx \
                                else hsb[:, k - kcx, bs]
                            for q in range(4):
                                nc.tensor.matmul(pss[q][:],
                                                 wg[:, k, q * P:(q + 1) * P],
                                                 rhs,
                                                 start=(k == 0),
                                                 stop=(k == n_k - 1))
                        # gates: q=0 i(sig), 1 f(sig), 2 g(tanh), 3 o(sig)
                        gt = []
                        for q in range(4):
                            func = AF.Tanh if q == 2 else AF.Sigmoid
                            t = gates_pool.tile([P, bw], f32, tag=f"g{q}",
                                                name=f"gate{q}_{l}_{j}_{b0}")
                            bcol = j * 4 + q
                            nc.scalar.activation(
                                t[:], pss[q][:], func,
                                bias=bias_sb[l][:, bcol:bcol + 1])
                            gt.append(t)
                        fc = ew_pool.tile([P, bw], f32, tag="fc",
                                          name=f"fc_{l}_{j}_{b0}")
                        nc.vector.tensor_mul(fc[:], gt[1][:], c_sb[:, bs])
                        ig = ew_pool.tile([P, bw], f32, tag="ig",
                                          name=f"ig_{l}_{j}_{b0}")
                        nc.vector.tensor_mul(ig[:], gt[0][:], gt[2][:])
                        cn = ew_pool.tile([P, bw], bf16, tag="cn",
                                          name=f"cn_{l}_{j}_{b0}")
                        nc.vector.tensor_add(cn[:], fc[:], ig[:])
                        # the last group's stores ride the sync queue (idle
                        # by then): a store issued from the scalar queue
                        # would delay the trailing tanh ACT by its issue time
                        steng = nc.sync if last_group else nc.scalar
                        steng.dma_start(cor[:, j, bs], cn[:])
                        th = ew_pool.tile([P, bw], f32, tag="th",
                                          name=f"th_{l}_{j}_{b0}")
                        nc.scalar.activation(th[:], cn[:], AF.Tanh)
                        hn = ew_pool.tile([P, bw], bf16, tag="hn",
                                          name=f"hn_{l}_{j}_{b0}")
                        nc.vector.tensor_mul(hn[:], gt[3][:], th[:])
                        steng.dma_start(hor[:, j, bs], hn[:])
    nc.compile()
    return nc


def _get_nc():
    if "nc" not in _CACHE:
        _CACHE["nc"] = _build_bass()
    return _CACHE["nc"]


# column permutation: new col (j*4+q)*128+t  <-  orig col q*1024+j*128+t
_PERM = np.arange(G4).reshape(4, NJ, P).transpose(1, 0, 2).reshape(-1)


def _prep_shared(inputs):
    """Host-side packing of the replicated weights.

    The tgt_emb linear layer is folded into cell1:
      in_frame @ w_ih1.T = kps @ (w_ih1 @ W_emb).T  (+ (w_ih1 @ b_emb))
    """
    f32 = np.float32

    def wT_perm(w):  # [G4, K] -> [K, G4] bf16, gate-interleaved columns
        return np.ascontiguousarray(w.T[:, _PERM]).astype(BF16)

    def b_pack(b):  # [G4] -> [128, 32] f32, col m' = j*4+q
        return np.ascontiguousarray(b[_PERM].reshape(4 * NJ, P).T).astype(f32)

    def group_pack(wxT, whT):
        # [K_tot, G4] -> [NJ, P, n_k, 512] so each h-chunk group is one
        # fully-contiguous DMA in the exact SBUF tile layout
        w_all = np.concatenate([wxT, whT], axis=0)
        n_k = w_all.shape[0] // P
        g = w_all.reshape(n_k, P, NJ, 4 * P).transpose(2, 1, 0, 3)
        return np.ascontiguousarray(g)

    w_ih1 = np.asarray(inputs["w_ih1"], f32)
    w_fused = w_ih1 @ np.asarray(inputs["W_emb"], f32)  # [G4, KPS]
    b1_fused = (inputs["b_ih1"] + inputs["b_hh1"]
                + w_ih1 @ np.asarray(inputs["b_emb"], f32))

    shared = {
        "wgrp0": group_pack(wT_perm(w_fused), wT_perm(inputs["w_hh1"])),
        "b0": b_pack(b1_fused),
    }
    for l, sfx in ((1, "2"), (2, "3")):
        shared[f"wgrp{l}"] = group_pack(wT_perm(inputs[f"w_ih{sfx}"]),
                                        wT_perm(inputs[f"w_hh{sfx}"]))
        shared[f"b{l}"] = b_pack(inputs[f"b_ih{sfx}"] + inputs[f"b_hh{sfx}"])
    return shared


def _pack_act(a):  # [B, F] -> [P, F//P, B] bf16 (p = f % 128, kc = f // 128)
    return np.ascontiguousarray(
        a.T.reshape(-1, P, a.shape[0]).transpose(1, 0, 2)).astype(BF16)


def _make_in_maps(inputs):
    shared = _prep_shared(inputs)
    in_maps = []
    for c in range(N_CORES):
        sl = slice(c * B, (c + 1) * B)
        m = dict(shared)
        m["kpsT"] = _pack_act(inputs["kps"][sl])
        for l in range(3):
            m[f"h{l}T"] = _pack_act(inputs[f"h{l}"][sl])
            m[f"c{l}T"] = np.ascontiguousarray(
                inputs[f"c{l}"][sl].T).astype(np.float32)
        in_maps.append(m)
    return in_maps


def _assemble(res):
    hn = [np.empty((B_FULL, HID), np.float32) for _ in range(3)]
    cn = [np.empty((B_FULL, HID), np.float32) for _ in range(3)]
    for c in range(N_CORES):
        sl = slice(c * B, (c + 1) * B)
        for l in range(3):
            hn[l][sl] = res[c][f"h{l}nT"].T.astype(np.float32)
            cn[l][sl] = res[c][f"c{l}nT"].T.astype(np.float32)
    return (hn[2], hn[0], hn[1], hn[2], cn[0], cn[1], cn[2])


def kernel(**inputs):
    from concourse.bass_utils import run_bass_kernel_spmd

    inputs = {k: np.asarray(v) for k, v in inputs.items()}
    nc = _get_nc()
    in_maps = _make_in_maps(inputs)
    res = run_bass_kernel_spmd(nc, in_maps, list(range(N_CORES))).results
    return _assemble(res)



# revision 16
# speedup vs baseline: 1.0032x; 1.0032x over previous
"""Trainium2 Bass kernel for a 3-cell LSTM decoder step (nn_AR_Decoder).

reference semantics (per sample):
    in_frame = kps @ W_emb.T + b_emb
    h0n,c0n = LSTMCell(in_frame, h0, c0; w_ih1,w_hh1,b_ih1,b_hh1)
    h1n,c1n = LSTMCell(h0,       h1, c1; w_ih2,w_hh2,b_ih2,b_hh2)
    h2n,c2n = LSTMCell(h1,       h2, c2; w_ih3,w_hh3,b_ih3,b_hh3)
    return (h2n, h0n, h1n, h2n, c0n, c1n, c2n)

Strategy: data-parallel over 8 NeuronCores (512 samples each). On device
everything lives in a transposed [feature, batch] layout so that
  * matmuls put the contraction dim (input features) on SBUF partitions,
    weights are the stationary operand, activations stream with N=512,
  * the per-gate bias is a per-partition scalar fused into the ACT pass,
  * no transposes are needed on device (host transposes in/out instead).
Weight columns are permuted on host so the 4 gates of one 128-row h-chunk
are contiguous; each group of 4 PSUM banks then yields i/f/g/o for one
h-chunk, which the DVE combines into c_new/h_new.
Matmuls run in bf16 (fp32 PSUM accumulation).

The kernel is PE-bound: 1344 matmuls x 512 moving rows = 287us at
2.4 GHz, so the whole game is keeping the PE gap-free from the first
instruction on.  Two HW behaviours dominate the schedule:
  * the PE clock ramps 0.65 -> 1.2 -> 2.4 GHz and reaches full speed
    only after ~3.4us of CONTINUOUS activity -- any DMA-starved gap
    resets the ramp, so the startup DMA order is arranged so the PE
    starts at ~1.5us and never stalls afterwards;
  * HAM grants roughly ~297us of full-rate PE activity before clamping
    utilization to 50%, so PE busy time (incl. any warm-up) must stay
    under that budget -- hence no dummy warm-up matmuls at all.
All activation/state tensors are host-packed as [128, kc, B] so every
DMA is fully contiguous per partition (1KB-segmented DMAs run at ~half
HBM bandwidth and would stretch the critical startup window).
"""

import sys

import numpy as np

for _p in ("/opt/trn_rl_repo", "/root/.axon_site/_ro/trn_rl_repo"):
    if _p not in sys.path:
        sys.path.append(_p)

import ml_dtypes

N_CORES = 8
B_FULL = 4096
B = B_FULL // N_CORES  # 512 per core
KPS = 256
HID = 1024
G4 = 4 * HID
P = 128
KC_KPS = KPS // P  # 2
KC_HID = HID // P  # 8
NJ = KC_HID  # 8 h-chunks per cell
BF16 = ml_dtypes.bfloat16

_CACHE = {}


def _build_bass():
    import concourse.tile as tile
    from concourse import bacc, mybir

    f32 = mybir.dt.float32
    bf16 = mybir.dt.bfloat16
    AF = mybir.ActivationFunctionType

    nc = bacc.Bacc("TRN2", target_bir_lowering=False, debug=False,
                   num_devices=N_CORES)

    # ---- DRAM parameters (per-core shapes) ----
    # activations/state come in pre-packed as [P, kc, B] (host does the
    # (kc p) b -> p kc b shuffle) so every load is one fully-contiguous
    # run per partition instead of kc separate 1KB segments.
    kpsT = nc.dram_tensor("kpsT", [P, KC_KPS, B], bf16,
                          kind="ExternalInput").ap()
    hT = [nc.dram_tensor(f"h{l}T", [P, KC_HID, B], bf16,
                         kind="ExternalInput").ap() for l in range(3)]
    cT = [nc.dram_tensor(f"c{l}T", [HID, B], f32, kind="ExternalInput").ap()
          for l in range(3)]
    kcx_l = [KC_KPS, KC_HID, KC_HID]
    # group-major weight layout: wgrp[j, p, k, :] holds the [128, n_k, 512]
    # weight block of h-chunk group j exactly as the SBUF tile wants it, so
    # each group is ONE fully-contiguous DMA (n_k*1KB per partition) instead
    # of n_k 1KB segments — ~2x DMA efficiency and a much shorter startup ramp
    wgrp = [nc.dram_tensor(f"wgrp{l}", [NJ, P, kcx_l[l] + KC_HID, 4 * P],
                           bf16, kind="ExternalInput").ap() for l in range(3)]
    bias = [nc.dram_tensor(f"b{l}", [P, 4 * NJ], f32,
                           kind="ExternalInput").ap() for l in range(3)]
    # outputs in bf16: halves store DMA bytes and the final write-receipt
    # latency on the critical tail (rel-err budget has 4x headroom)
    hoT = [nc.dram_tensor(f"h{l}nT", [HID, B], bf16, kind="ExternalOutput").ap()
           for l in range(3)]
    coT = [nc.dram_tensor(f"c{l}nT", [HID, B], bf16, kind="ExternalOutput").ap()
           for l in range(3)]

    with tile.TileContext(nc) as tc:
        with (
            tc.tile_pool(name="acts", bufs=1) as acts_pool,
            tc.tile_pool(name="wpool", bufs=3) as wpool,
            tc.tile_pool(name="cpool", bufs=3) as cpool,
            tc.tile_pool(name="gates", bufs=2) as gates_pool,
            tc.tile_pool(name="ew", bufs=2) as ew_pool,
            tc.tile_pool(name="psum", bufs=8, space="PSUM") as psum_pool,
        ):
            # ---- persistent SBUF tiles ----
            kps_sb = acts_pool.tile([P, KC_KPS, B], bf16)
            h_sb = []
            for l in range(3):
                t = acts_pool.tile([P, KC_HID, B], bf16, name=f"h{l}sb")
                h_sb.append(t)
            bias_sb = [acts_pool.tile([P, 4 * NJ], f32, name=f"b{l}sb")
                       for l in range(3)]

            # ---- HAM warm-up during the startup DMA window ----
            # The exec-time window opens ~5us before the first weight bytes
            # can land (NTFF first_useful pins to the framework memsets, and
            # the first DMA has ~1.5us HWDGE first-byte latency).  The PE is
            # idle in that window, so full-duty dummy matmuls on a garbage
            # SBUF tile are FREE there and start the HAM busy streak ~5us
            # earlier: the 8/8 grant (86 epochs) fires around the time the
            # first real matmul issues instead of ~4.5us into the real
            # stream, saving ~2us of half-clock tax.  (The earlier "net
            # zero" warm-up experiment gated the dummies on the kps arrival,
            # so they ran INSIDE the real stream and displaced real matmuls
            # 1:1; these run in otherwise-dead time.)
            warm_in = acts_pool.tile([P, P], bf16, name="warm_in")
            # gpsimd is idle at body start; the tile must be written once
            # before the PE may read it (tile allocation = first write)
            nc.gpsimd.memset(warm_in[:], 0.0)
            warm_ps = [psum_pool.tile([P, B], f32, tag="ps", name=f"warmps{q}")
                       for q in range(4)]
            for i in range(36):
                # 128-col dummies (107ns each cold) until ~the time the
                # first real matmul's data can have landed (~11.3us): keeps
                # the HAM busy streak unbroken from ~7.5us so the 8/8 grant
                # fires around the real stream start instead of ~4.5us in.
                nc.tensor.matmul(warm_ps[i % 4][:, 0:P], warm_in[:],
                                 warm_in[:], start=True, stop=True)

            # ---- startup DMA: two parallel HWDGE queues ----
            # Early supply is ~220 GB/s per queue (transfer-serialized on
            # each queue), while a WARM PE in group j0 consumes weights + x
            # at ~300 GB/s.  So the two streams are split: weights/kps/c on
            # SYNC, the h0 quarters (j0's x chunks) + biases + h1/h2 on
            # SCALAR.  Modeled piece-arrival vs consumption then has no
            # stalls: kps+wg(k0:2) land ~11.1us, each later piece lands
            # >=1us before the warm PE reaches it, and h1/h2 finish by
            # ~25us, long before cell 2 needs them at ~100us.
            nc.sync.dma_start(kps_sb[:], kpsT)
            nc.scalar.dma_start(bias_sb[0][:], bias[0])
            for q in range(4):
                nc.scalar.dma_start(h_sb[0][:, 2 * q:2 * q + 2, :],
                                    hT[0][:, 2 * q:2 * q + 2, :])
            nc.scalar.dma_start(bias_sb[1][:], bias[1])
            nc.scalar.dma_start(bias_sb[2][:], bias[2])
            nc.scalar.dma_start(h_sb[1][:], hT[1])
            nc.scalar.dma_start(h_sb[2][:], hT[2])

            # ---- the 3 LSTM cells ----
            # (x source tile, #x k-chunks, h source tile) per cell
            specs = [
                (kps_sb, KC_KPS, h_sb[0]),
                (h_sb[0], KC_HID, h_sb[1]),
                (h_sb[1], KC_HID, h_sb[2]),
            ]
            for l, (xsb, kcx, hsb) in enumerate(specs):
                n_k = kcx + KC_HID
                cr = cT[l].rearrange("(kc p) b -> p kc b", p=P)
                hor = hoT[l].rearrange("(kc p) b -> p kc b", p=P)
                cor = coT[l].rearrange("(kc p) b -> p kc b", p=P)
                for j in range(NJ):
                    wg = wpool.tile([P, n_k, 4 * P], bf16, tag="wg",
                                    name=f"wg_{l}_{j}")
                    if l == 0 and j == 0:
                        # j0 in 2-k-chunk pieces (the h0 quarters ride the
                        # scalar queue in parallel): the PE starts on k0/k1
                        # (the kps chunks) after ~0.5MB of sync DMA and each
                        # later piece lands well before the PE reaches it,
                        # so the clock ramp is never reset by a stall.
                        nc.sync.dma_start(wg[:, 0:2, :], wgrp[l][j, :, 0:2, :])
                        for q in range(4):
                            nc.sync.dma_start(
                                wg[:, 2 * q + 2:2 * q + 4, :],
                                wgrp[l][j, :, 2 * q + 2:2 * q + 4, :])
                    elif l == 0 and j == 1:
                        # j1 front-loaded in small pieces too: its first
                        # k-chunks are needed ~1.5us before the whole-group
                        # DMA could have finished.
                        nc.sync.dma_start(wg[:, 0:2, :], wgrp[l][j, :, 0:2, :])
                        nc.sync.dma_start(wg[:, 2:4, :], wgrp[l][j, :, 2:4, :])
                        nc.sync.dma_start(wg[:, 4:, :], wgrp[l][j, :, 4:, :])
                    elif l == 0 and j == 2:
                        # j2 in halves: the warm PE reaches j2 with only
                        # ~1us of supply margin; half-group completion
                        # granularity keeps it fed.
                        nc.sync.dma_start(wg[:, 0:5, :], wgrp[l][j, :, 0:5, :])
                        nc.sync.dma_start(wg[:, 5:, :], wgrp[l][j, :, 5:, :])
                    else:
                        nc.sync.dma_start(wg[:], wgrp[l][j])
                    # c rides the same FIFO right behind its group's weights
                    # (needed ~13us later, so never on the critical path, but
                    # on any other queue it would float to kernel start)
                    c_sb = cpool.tile([P, B], f32, tag="c", name=f"c_{l}_{j}")
                    nc.sync.dma_start(c_sb[:], cr[:, j, :])

                    # the very last group is split into two batch halves so
                    # its ACT/DVE/store chain overlaps the second half's
                    # matmuls instead of all trailing after the final MM.
                    # Exactly two halves is the measured optimum: no split
                    # costs ~1.8us (tail after the final matmul adds to
                    # exec 1:1), while half+quarter+quarter costs ~1us
                    # (extra instruction-fetch pressure + smaller tiles).
                    last_group = (l == 2 and j == NJ - 1)
                    parts = [(0, B // 2), (B // 2, B // 2)] if last_group \
                        else [(0, B)]
                    for b0, bw in parts:
                        bs = slice(b0, b0 + bw)
                        pss = [psum_pool.tile([P, bw], f32, tag="ps",
                                              name=f"ps{q}_{l}_{j}_{b0}")
                               for q in range(4)]
                        # k-outer / q-inner: consecutive matmuls rotate
                        # across 4 PSUM banks, hiding the PE drain-to-PSUM
                        # latency (same-bank back-to-back accumulation costs
                        # ~43ns/MM extra).
                        for k in range(n_k):
                            rhs = xsb[:, k, bs] if k < kcx \
                                else hsb[:, k - kcx, bs]
                            for q in range(4):
                                nc.tensor.matmul(pss[q][:],
                                                 wg[:, k, q * P:(q + 1) * P],
                                                 rhs,
                                                 start=(k == 0),
                                                 stop=(k == n_k - 1))
                        # gates: q=0 i(sig), 1 f(sig), 2 g(tanh), 3 o(sig)
                        gt = []
                        for q in range(4):
                            func = AF.Tanh if q == 2 else AF.Sigmoid
                            t = gates_pool.tile([P, bw], f32, tag=f"g{q}",
                                                name=f"gate{q}_{l}_{j}_{b0}")
                            bcol = j * 4 + q
                            nc.scalar.activation(
                                t[:], pss[q][:], func,
                                bias=bias_sb[l][:, bcol:bcol + 1])
                            gt.append(t)
                        fc = ew_pool.tile([P, bw], f32, tag="fc",
                                          name=f"fc_{l}_{j}_{b0}")
                        nc.vector.tensor_mul(fc[:], gt[1][:], c_sb[:, bs])
                        ig = ew_pool.tile([P, bw], f32, tag="ig",
                                          name=f"ig_{l}_{j}_{b0}")
                        nc.vector.tensor_mul(ig[:], gt[0][:], gt[2][:])
                        cn = ew_pool.tile([P, bw], bf16, tag="cn",
                                          name=f"cn_{l}_{j}_{b0}")
                        nc.vector.tensor_add(cn[:], fc[:], ig[:])
                        # the last group's stores ride the sync queue (idle
                        # by then): a store issued from the scalar queue
                        # would delay the trailing tanh ACT by its issue time
                        steng = nc.sync if last_group else nc.scalar
                        steng.dma_start(cor[:, j, bs], cn[:])
                        th = ew_pool.tile([P, bw], f32, tag="th",
                                          name=f"th_{l}_{j}_{b0}")
                        nc.scalar.activation(th[:], cn[:], AF.Tanh)
                        hn = ew_pool.tile([P, bw], bf16, tag="hn",
                                          name=f"hn_{l}_{j}_{b0}")
                        nc.vector.tensor_mul(hn[:], gt[3][:], th[:])
                        steng.dma_start(hor[:, j, bs], hn[:])
    nc.compile()
    return nc


def _get_nc():
    if "nc" not in _CACHE:
        _CACHE["nc"] = _build_bass()
    return _CACHE["nc"]


# column permutation: new col (j*4+q)*128+t  <-  orig col q*1024+j*128+t
_PERM = np.arange(G4).reshape(4, NJ, P).transpose(1, 0, 2).reshape(-1)


def _prep_shared(inputs):
    """Host-side packing of the replicated weights.

    The tgt_emb linear layer is folded into cell1:
      in_frame @ w_ih1.T = kps @ (w_ih1 @ W_emb).T  (+ (w_ih1 @ b_emb))
    """
    f32 = np.float32

    def wT_perm(w):  # [G4, K] -> [K, G4] bf16, gate-interleaved columns
        return np.ascontiguousarray(w.T[:, _PERM]).astype(BF16)

    def b_pack(b):  # [G4] -> [128, 32] f32, col m' = j*4+q
        return np.ascontiguousarray(b[_PERM].reshape(4 * NJ, P).T).astype(f32)

    def group_pack(wxT, whT):
        # [K_tot, G4] -> [NJ, P, n_k, 512] so each h-chunk group is one
        # fully-contiguous DMA in the exact SBUF tile layout
        w_all = np.concatenate([wxT, whT], axis=0)
        n_k = w_all.shape[0] // P
        g = w_all.reshape(n_k, P, NJ, 4 * P).transpose(2, 1, 0, 3)
        return np.ascontiguousarray(g)

    w_ih1 = np.asarray(inputs["w_ih1"], f32)
    w_fused = w_ih1 @ np.asarray(inputs["W_emb"], f32)  # [G4, KPS]
    b1_fused = (inputs["b_ih1"] + inputs["b_hh1"]
                + w_ih1 @ np.asarray(inputs["b_emb"], f32))

    shared = {
        "wgrp0": group_pack(wT_perm(w_fused), wT_perm(inputs["w_hh1"])),
        "b0": b_pack(b1_fused),
    }
    for l, sfx in ((1, "2"), (2, "3")):
        shared[f"wgrp{l}"] = group_pack(wT_perm(inputs[f"w_ih{sfx}"]),
                                        wT_perm(inputs[f"w_hh{sfx}"]))
        shared[f"b{l}"] = b_pack(inputs[f"b_ih{sfx}"] + inputs[f"b_hh{sfx}"])
    return shared


def _pack_act(a):  # [B, F] -> [P, F//P, B] bf16 (p = f % 128, kc = f // 128)
    return np.ascontiguousarray(
        a.T.reshape(-1, P, a.shape[0]).transpose(1, 0, 2)).astype(BF16)


def _make_in_maps(inputs):
    shared = _prep_shared(inputs)
    in_maps = []
    for c in range(N_CORES):
        sl = slice(c * B, (c + 1) * B)
        m = dict(shared)
        m["kpsT"] = _pack_act(inputs["kps"][sl])
        for l in range(3):
            m[f"h{l}T"] = _pack_act(inputs[f"h{l}"][sl])
            m[f"c{l}T"] = np.ascontiguousarray(
                inputs[f"c{l}"][sl].T).astype(np.float32)
        in_maps.append(m)
    return in_maps


def _assemble(res):
    hn = [np.empty((B_FULL, HID), np.float32) for _ in range(3)]
    cn = [np.empty((B_FULL, HID), np.float32) for _ in range(3)]
    for c in range(N_CORES):
        sl = slice(c * B, (c + 1) * B)
        for l in range(3):
            hn[l][sl] = res[c][f"h{l}nT"].T.astype(np.float32)
            cn[l][sl] = res[c][f"c{l}nT"].T.astype(np.float32)
    return (hn[2], hn[0], hn[1], hn[2], cn[0], cn[1], cn[2])


def kernel(**inputs):
    from concourse.bass_utils import run_bass_kernel_spmd

    inputs = {k: np.asarray(v) for k, v in inputs.items()}
    nc = _get_nc()
    in_maps = _make_in_maps(inputs)
    res = run_bass_kernel_spmd(nc, in_maps, list(range(N_CORES))).results
    return _assemble(res)



# revision 17
# speedup vs baseline: 1.0179x; 1.0147x over previous
"""Trainium2 Bass kernel for a 3-cell LSTM decoder step (nn_AR_Decoder).

reference semantics (per sample):
    in_frame = kps @ W_emb.T + b_emb
    h0n,c0n = LSTMCell(in_frame, h0, c0; w_ih1,w_hh1,b_ih1,b_hh1)
    h1n,c1n = LSTMCell(h0,       h1, c1; w_ih2,w_hh2,b_ih2,b_hh2)
    h2n,c2n = LSTMCell(h1,       h2, c2; w_ih3,w_hh3,b_ih3,b_hh3)
    return (h2n, h0n, h1n, h2n, c0n, c1n, c2n)

Strategy: data-parallel over 8 NeuronCores (512 samples each). On device
everything lives in a transposed [feature, batch] layout so that
  * matmuls put the contraction dim (input features) on SBUF partitions,
    weights are the stationary operand, activations stream with N=512,
  * the per-gate bias is a per-partition scalar fused into the ACT pass,
  * no transposes are needed on device (host transposes in/out instead).
Weight columns are permuted on host so the 4 gates of one 128-row h-chunk
are contiguous; each group of 4 PSUM banks then yields i/f/g/o for one
h-chunk, which the DVE combines into c_new/h_new.
Matmuls run in bf16 (fp32 PSUM accumulation); h/c outputs are stored in
bf16 (the rel-err budget has 4x headroom and it halves store traffic).

The kernel is PE-bound: 1344 matmul-equivalents x 512 moving rows =
287us at 2.4 GHz, so the whole game is keeping the PE gap-free from the
first instruction on.  Hard-won facts from NTFF traces:
  * the PE clock ramps 1.2 -> 2.4 GHz; HAM un-throttles only after ~3.4us
    of CONTINUOUS activity (epoch phase is free-running, so the fire time
    varies run to run), and the 8/8 grant lasts exactly 86 epochs
    (~293.5us) -- the PE stream must fit inside it;
  * early DMA supply is ~220 GB/s on one HWDGE queue and each dma_start
    costs ~0.7us of issue time regardless of size, so during group j0 a
    WARM PE (~300 GB/s consumption) starves, breaks the HAM streak, and
    re-throttles (catastrophic).  The cold-clock start is almost exactly
    supply-matched, so warm-up dummies CANNOT help (measured: every
    warm-start variant lost 1.3-4.8us to starvation stalls + HAM
    oscillation).  Startup order just minimizes the first-matmul gate:
    k0-only first pieces (262KB) instead of k0:2+kps (518KB);
  * the tail is latency-bound: 3 serial gate ACTs + DVE chain + tanh +
    store + ~1.7us HBM write receipt after the last matmul.  The last
    group's second half is therefore processed "gates-split": i/f/g
    matmuls first, then the whole c_new/tanh chain runs WHILE the o-gate
    matmuls stream, leaving only ACT(o) + mul + store after the last MM.
All activation/state tensors are host-packed as [128, kc, B] so every
DMA is fully contiguous per partition (1KB-segmented DMAs run at ~half
HBM bandwidth and would stretch the critical startup window).
"""

import sys

import numpy as np

for _p in ("/opt/trn_rl_repo", "/root/.axon_site/_ro/trn_rl_repo"):
    if _p not in sys.path:
        sys.path.append(_p)

import ml_dtypes

N_CORES = 8
B_FULL = 4096
B = B_FULL // N_CORES  # 512 per core
KPS = 256
HID = 1024
G4 = 4 * HID
P = 128
KC_KPS = KPS // P  # 2
KC_HID = HID // P  # 8
NJ = KC_HID  # 8 h-chunks per cell
BF16 = ml_dtypes.bfloat16

_CACHE = {}


def _build_bass():
    import concourse.tile as tile
    from concourse import bacc, mybir

    f32 = mybir.dt.float32
    bf16 = mybir.dt.bfloat16
    AF = mybir.ActivationFunctionType

    nc = bacc.Bacc("TRN2", target_bir_lowering=False, debug=False,
                   num_devices=N_CORES)

    # ---- DRAM parameters (per-core shapes) ----
    # activations/state come in pre-packed as [P, kc, B] (host does the
    # (kc p) b -> p kc b shuffle) so every load is one fully-contiguous
    # run per partition instead of kc separate 1KB segments.
    kpsT = nc.dram_tensor("kpsT", [P, KC_KPS, B], bf16,
                          kind="ExternalInput").ap()
    hT = [nc.dram_tensor(f"h{l}T", [P, KC_HID, B], bf16,
                         kind="ExternalInput").ap() for l in range(3)]
    cT = [nc.dram_tensor(f"c{l}T", [HID, B], f32, kind="ExternalInput").ap()
          for l in range(3)]
    kcx_l = [KC_KPS, KC_HID, KC_HID]
    # group-major weight layout: wgrp[j, p, k, :] holds the [128, n_k, 512]
    # weight block of h-chunk group j exactly as the SBUF tile wants it, so
    # each group is ONE fully-contiguous DMA (n_k*1KB per partition) instead
    # of n_k 1KB segments — ~2x DMA efficiency and a much shorter startup ramp
    wgrp = [nc.dram_tensor(f"wgrp{l}", [NJ, P, kcx_l[l] + KC_HID, 4 * P],
                           bf16, kind="ExternalInput").ap() for l in range(3)]
    bias = [nc.dram_tensor(f"b{l}", [P, 4 * NJ], f32,
                           kind="ExternalInput").ap() for l in range(3)]
    # outputs in bf16: halves store DMA traffic (rel-err budget has 4x
    # headroom; host casts back to f32)
    hoT = [nc.dram_tensor(f"h{l}nT", [HID, B], bf16, kind="ExternalOutput").ap()
           for l in range(3)]
    coT = [nc.dram_tensor(f"c{l}nT", [HID, B], bf16, kind="ExternalOutput").ap()
           for l in range(3)]

    with tile.TileContext(nc) as tc:
        with (
            tc.tile_pool(name="acts", bufs=1) as acts_pool,
            tc.tile_pool(name="wpool", bufs=3) as wpool,
            tc.tile_pool(name="cpool", bufs=3) as cpool,
            tc.tile_pool(name="gates", bufs=2) as gates_pool,
            tc.tile_pool(name="ew", bufs=2) as ew_pool,
            tc.tile_pool(name="psum", bufs=8, space="PSUM") as psum_pool,
        ):
            # ---- persistent SBUF tiles ----
            kps_sb = acts_pool.tile([P, KC_KPS, B], bf16)
            h_sb = []
            for l in range(3):
                t = acts_pool.tile([P, KC_HID, B], bf16, name=f"h{l}sb")
                h_sb.append(t)
            bias_sb = []
            for l in range(3):
                t = acts_pool.tile([P, 4 * NJ], f32, name=f"b{l}sb")
                if l == 0:
                    nc.scalar.dma_start(t[:], bias[l])
                bias_sb.append(t)

            # ---- the 3 LSTM cells ----
            # (x source tile, #x k-chunks, h source tile) per cell
            specs = [
                (kps_sb, KC_KPS, h_sb[0]),
                (h_sb[0], KC_HID, h_sb[1]),
                (h_sb[1], KC_HID, h_sb[2]),
            ]
            for l, (xsb, kcx, hsb) in enumerate(specs):
                n_k = kcx + KC_HID
                cr = cT[l].rearrange("(kc p) b -> p kc b", p=P)
                hor = hoT[l].rearrange("(kc p) b -> p kc b", p=P)
                cor = coT[l].rearrange("(kc p) b -> p kc b", p=P)
                if l > 0:
                    # deferred loads: cell l's h-state and bias, positioned
                    # in the sync FIFO just ahead of this cell's weights
                    nc.sync.dma_start(h_sb[l][:], hT[l])
                    nc.scalar.dma_start(bias_sb[l][:], bias[l])
                for j in range(NJ):
                    wg = wpool.tile([P, n_k, 4 * P], bf16, tag="wg",
                                    name=f"wg_{l}_{j}")
                    if l == 0 and j == 0:
                        # startup: the very first matmul needs only wg k0 +
                        # kps k0 (262KB), so those land first as single-
                        # chunk pieces; h0 quarters are interleaved right
                        # before the k-chunks that consume them.  One queue,
                        # strict FIFO: supply is issue-rate bound (~0.7us
                        # per piece) so pieces stay >=256KB after the gate.
                        nc.sync.dma_start(wg[:, 0:1, :], wgrp[l][j, :, 0:1, :])
                        nc.sync.dma_start(kps_sb[:, 0:1, :], kpsT[:, 0:1, :])
                        nc.sync.dma_start(wg[:, 1:2, :], wgrp[l][j, :, 1:2, :])
                        nc.sync.dma_start(kps_sb[:, 1:2, :], kpsT[:, 1:2, :])
                        for q in range(4):
                            nc.sync.dma_start(h_sb[0][:, 2 * q:2 * q + 2, :],
                                              hT[0][:, 2 * q:2 * q + 2, :])
                            nc.sync.dma_start(
                                wg[:, 2 * q + 2:2 * q + 4, :],
                                wgrp[l][j, :, 2 * q + 2:2 * q + 4, :])
                    elif l == 0 and j == 1:
                        # j1 front-loaded in small pieces too: its first
                        # k-chunks are needed ~1.5us before the whole-group
                        # DMA could have finished.
                        nc.sync.dma_start(wg[:, 0:2, :], wgrp[l][j, :, 0:2, :])
                        nc.sync.dma_start(wg[:, 2:4, :], wgrp[l][j, :, 2:4, :])
                        nc.sync.dma_start(wg[:, 4:, :], wgrp[l][j, :, 4:, :])
                    else:
                        nc.sync.dma_start(wg[:], wgrp[l][j])
                    # c rides the same FIFO right behind its group's weights
                    # (needed ~13us later, so never on the critical path, but
                    # on any other queue it would float to kernel start)
                    c_sb = cpool.tile([P, B], f32, tag="c", name=f"c_{l}_{j}")
                    nc.sync.dma_start(c_sb[:], cr[:, j, :])

                    # the very last group is split into two batch halves so
                    # its ACT/DVE/store chain overlaps the second half's
                    # matmuls instead of all trailing after the final MM.
                    # The second half additionally runs gates-split (see
                    # below).
                    last_group = (l == 2 and j == NJ - 1)
                    parts = [(0, B // 2), (B // 2, B // 2)] if last_group \
                        else [(0, B)]
                    for b0, bw in parts:
                        bs = slice(b0, b0 + bw)
                        split_gates = last_group and b0 > 0
                        pss = [psum_pool.tile([P, bw], f32, tag="ps",
                                              name=f"ps{q}_{l}_{j}_{b0}")
                               for q in range(4)]
                        # k-outer / q-inner: consecutive matmuls rotate
                        # across PSUM banks, hiding the PE drain-to-PSUM
                        # latency (same-bank back-to-back accumulation costs
                        # ~43ns/MM extra).
                        qs_a = (0, 1, 2) if split_gates else (0, 1, 2, 3)
                        for k in range(n_k):
                            rhs = xsb[:, k, bs] if k < kcx \
                                else hsb[:, k - kcx, bs]
                            for q in qs_a:
                                nc.tensor.matmul(pss[q][:],
                                                 wg[:, k, q * P:(q + 1) * P],
                                                 rhs,
                                                 start=(k == 0),
                                                 stop=(k == n_k - 1))
                        # gates: q=0 i(sig), 1 f(sig), 2 g(tanh), 3 o(sig)
                        gt = []
                        for q in qs_a:
                            func = AF.Tanh if q == 2 else AF.Sigmoid
                            t = gates_pool.tile([P, bw], f32, tag=f"g{q}",
                                                name=f"gate{q}_{l}_{j}_{b0}")
                            bcol = j * 4 + q
                            nc.scalar.activation(
                                t[:], pss[q][:], func,
                                bias=bias_sb[l][:, bcol:bcol + 1])
                            gt.append(t)
                        fc = ew_pool.tile([P, bw], f32, tag="fc",
                                          name=f"fc_{l}_{j}_{b0}")
                        nc.vector.tensor_mul(fc[:], gt[1][:], c_sb[:, bs])
                        ig = ew_pool.tile([P, bw], f32, tag="ig",
                                          name=f"ig_{l}_{j}_{b0}")
                        nc.vector.tensor_mul(ig[:], gt[0][:], gt[2][:])
                        cn = ew_pool.tile([P, bw], bf16, tag="cn",
                                          name=f"cn_{l}_{j}_{b0}")
                        nc.vector.tensor_add(cn[:], fc[:], ig[:])
                        # the last group's stores ride the sync queue (idle
                        # by then): a store issued from the scalar queue
                        # would delay the trailing tanh ACT by its issue time
                        steng = nc.sync if last_group else nc.scalar
                        steng.dma_start(cor[:, j, bs], cn[:])
                        th = ew_pool.tile([P, bw], f32, tag="th",
                                          name=f"th_{l}_{j}_{b0}")
                        nc.scalar.activation(th[:], cn[:], AF.Tanh)
                        if split_gates:
                            # B-pass: the o-gate matmuls stream WHILE the
                            # c_new/tanh chain above executes, so the
                            # post-last-MM tail is only ACT(o)+mul+store.
                            # Single-bank accumulation (the ~43ns same-bank
                            # penalty is cheaper than an extra combine op).
                            for k in range(n_k):
                                rhs = xsb[:, k, bs] if k < kcx \
                                    else hsb[:, k - kcx, bs]
                                nc.tensor.matmul(pss[3][:],
                                                 wg[:, k, 3 * P:4 * P],
                                                 rhs,
                                                 start=(k == 0),
                                                 stop=(k == n_k - 1))
                            t_o = gates_pool.tile([P, bw], f32, tag="g3",
                                                  name=f"gate3_{l}_{j}_{b0}")
                            nc.scalar.activation(
                                t_o[:], pss[3][:], AF.Sigmoid,
                                bias=bias_sb[l][:, j * 4 + 3:j * 4 + 4])
                            gt.append(t_o)
                        hn = ew_pool.tile([P, bw], bf16, tag="hn",
                                          name=f"hn_{l}_{j}_{b0}")
                        nc.vector.tensor_mul(hn[:], gt[3][:], th[:])
                        steng.dma_start(hor[:, j, bs], hn[:])
    nc.compile()
    return nc


def _get_nc():
    if "nc" not in _CACHE:
        _CACHE["nc"] = _build_bass()
    return _CACHE["nc"]


# column permutation: new col (j*4+q)*128+t  <-  orig col q*1024+j*128+t
_PERM = np.arange(G4).reshape(4, NJ, P).transpose(1, 0, 2).reshape(-1)


def _prep_shared(inputs):
    """Host-side packing of the replicated weights.

    The tgt_emb linear layer is folded into cell1:
      in_frame @ w_ih1.T = kps @ (w_ih1 @ W_emb).T  (+ (w_ih1 @ b_emb))
    """
    f32 = np.float32

    def wT_perm(w):  # [G4, K] -> [K, G4] bf16, gate-interleaved columns
        return np.ascontiguousarray(w.T[:, _PERM]).astype(BF16)

    def b_pack(b):  # [G4] -> [128, 32] f32, col m' = j*4+q
        return np.ascontiguousarray(b[_PERM].reshape(4 * NJ, P).T).astype(f32)

    def group_pack(wxT, whT):
        # [K_tot, G4] -> [NJ, P, n_k, 512] so each h-chunk group is one
        # fully-contiguous DMA in the exact SBUF tile layout
        w_all = np.concatenate([wxT, whT], axis=0)
        n_k = w_all.shape[0] // P
        g = w_all.reshape(n_k, P, NJ, 4 * P).transpose(2, 1, 0, 3)
        return np.ascontiguousarray(g)

    w_ih1 = np.asarray(inputs["w_ih1"], f32)
    w_fused = w_ih1 @ np.asarray(inputs["W_emb"], f32)  # [G4, KPS]
    b1_fused = (inputs["b_ih1"] + inputs["b_hh1"]
                + w_ih1 @ np.asarray(inputs["b_emb"], f32))

    shared = {
        "wgrp0": group_pack(wT_perm(w_fused), wT_perm(inputs["w_hh1"])),
        "b0": b_pack(b1_fused),
    }
    for l, sfx in ((1, "2"), (2, "3")):
        shared[f"wgrp{l}"] = group_pack(wT_perm(inputs[f"w_ih{sfx}"]),
                                        wT_perm(inputs[f"w_hh{sfx}"]))
        shared[f"b{l}"] = b_pack(inputs[f"b_ih{sfx}"] + inputs[f"b_hh{sfx}"])
    return shared


def _pack_act(a):  # [B, F] -> [P, F//P, B] bf16 (p = f % 128, kc = f // 128)
    return np.ascontiguousarray(
        a.T.reshape(-1, P, a.shape[0]).transpose(1, 0, 2)).astype(BF16)


def _make_in_maps(inputs):
    shared = _prep_shared(inputs)
    in_maps = []
    for c in range(N_CORES):
        sl = slice(c * B, (c + 1) * B)
        m = dict(shared)
        m["kpsT"] = _pack_act(inputs["kps"][sl])
        for l in range(3):
            m[f"h{l}T"] = _pack_act(inputs[f"h{l}"][sl])
            m[f"c{l}T"] = np.ascontiguousarray(
                inputs[f"c{l}"][sl].T).astype(np.float32)
        in_maps.append(m)
    return in_maps


def _assemble(res):
    hn = [np.empty((B_FULL, HID), np.float32) for _ in range(3)]
    cn = [np.empty((B_FULL, HID), np.float32) for _ in range(3)]
    for c in range(N_CORES):
        sl = slice(c * B, (c + 1) * B)
        for l in range(3):
            hn[l][sl] = res[c][f"h{l}nT"].T.astype(np.float32)
            cn[l][sl] = res[c][f"c{l}nT"].T.astype(np.float32)
    return (hn[2], hn[0], hn[1], hn[2], cn[0], cn[1], cn[2])


def kernel(**inputs):
    from concourse.bass_utils import run_bass_kernel_spmd

    inputs = {k: np.asarray(v) for k, v in inputs.items()}
    nc = _get_nc()
    in_maps = _make_in_maps(inputs)
    res = run_bass_kernel_spmd(nc, in_maps, list(range(N_CORES))).results
    return _assemble(res)
